# revision 44
# baseline (speedup 1.0000x reference)
"""GCN layer kernel for Trainium2, 8-core SPMD.

Computes: out = (A @ (X @ W + b)) / colsum(A)[:, None],  A = (adj != 0)
with N=8192 nodes, F_in=F_out=512, across 8 NeuronCores.

Sharding: row-shard adjacency and node features (1024 rows per core),
replicate W/b. Degree (column sums of A) needs rows from every core: each
core computes partial column sums for free via the binarize op's
accumulate output, an all-gather shares them, and an on-device tree sum
+ reciprocal finishes the normalization.

All tensor inputs are shipped to HBM as bf16 (host-side dtype cast only:
the (adj != 0) predicate is exactly preserved by the cast since no
uniform[0,1) float rounds to 0 in bf16, and X/W/b are converted to bf16
on-device by the reference-equivalent compute path anyway). This halves
the dominant A-stream HBM traffic (32 -> 16 MB/core). Binarize + degree
+ projection + aggregation + normalization all still happen on device.

Measured-on-hw notes driving the structure:
- bf16 [128,512] matmuls: 454ns latency, ~265ns pipelined throughput
  (~1.95GHz effective). Main loop floor = 512 mm ~= 136us.
- The CC engine cannot enter a collective before ~20.5us and every
  collective pays a ~45us rendezvous after entry (NOT amortizable by a
  warm-up collective - that only queues ahead and delays the H gather).
  At ~160GB/s cc wire rate the H AllGather completes ~105-125us in.
- Each core computes H = X@W+b for itself and the next LR-1 ranks from
  cheap extra X inputs; the main loop visits contraction tiles in
  per-core rotated order kt=(pid*8+i)%64, so its first LR*8 iterations
  use local H blocks and cover the whole gather window; only the far
  tiles read gathered data.
- bias lands via a K=1 matmul per H tile; evacuation is an ACT copy.
  Keeping evacuations OFF the DVE matters: the static scheduler
  otherwise interleaves them with binarizes into a convoy that stalls
  the pipeline behind the collective.
- cs partials are stored in ITERATION order and bins/mms/araws keep a
  single monotonic order - reordering them (canonical columns,
  sub-major far order) makes the scheduler's coarsened semaphore waits
  couple PE matmuls to much-later binarizes, deadlocking the pipeline
  until the collective lands. The degree combine rotates per-rank
  slices to compensate.
"""
import numpy as np

N = 8192
F = 512
N_CORES = 8
NB = N // N_CORES          # 1024 rows per core
KT = N // 128              # 64 contraction tiles
MT = NB // 128             # 8 output row tiles per core
FI_T = F // 128            # 4 feat-in tiles
LR = 5                     # ranks whose H we compute locally
RBUF = 6                   # a_raw ring depth (DMA run-ahead)
ABUF = 30                  # a_bin ring depth (DVE run-ahead); deep so
                           # the binarizes (and the degree partials they
                           # accumulate) finish well before the PE does,
                           # keeping the degree AllGather off the tail
HBUF = 16                  # gathered-H tile ring depth
PRE_BIN = 3                # binarizes interleaved per H rank block

_cached = {}


def _build():
    import concourse.bacc as bacc
    import concourse.bass as bass
    import concourse.tile as tile
    from concourse import mybir

    f32 = mybir.dt.float32
    bf16 = mybir.dt.bfloat16

    nc = bacc.Bacc("TRN2", target_bir_lowering=False, debug=False,
                   num_devices=N_CORES)
    at = nc.dram_tensor("at", [N, NB], bf16, kind="ExternalInput").ap()
    xt = nc.dram_tensor("xt", [F, LR * NB], bf16, kind="ExternalInput").ap()
    w = nc.dram_tensor("w", [F, F], bf16, kind="ExternalInput").ap()
    bvec = nc.dram_tensor("bvec", [1, F], bf16, kind="ExternalInput").ap()
    out = nc.dram_tensor("out", [NB, F], f32, kind="ExternalOutput").ap()

    pid = nc.partition_id()

    with tile.TileContext(nc) as tc:
        with tc.tile_pool(name="dram", bufs=1, space="DRAM") as dram, \
             tc.tile_pool(name="p", bufs=1) as p, \
             tc.tile_pool(name="ps", bufs=1, space="PSUM") as ps:
            hg_in = dram.tile([NB, F], bf16)
            hg_out = dram.tile([N, F], bf16, addr_space="Shared")
            dg_in = dram.tile([128, KT], f32)
            dg_out = dram.tile([128 * N_CORES, KT], f32, addr_space="Shared")

            # ---- critical-path DMAs first: b, W chunk 0, rank-0 X chunks
            b_bf = p.tile([1, F], bf16)
            nc.scalar.dma_start(b_bf[:], bvec)
            w_c = p.tile([128, FI_T * F], bf16)
            nc.scalar.dma_start(w_c[:, 0:F], w[0:128, :])
            # rank-0 X chunks ride the (otherwise idle) sync queue so the
            # first H matmul is not serialized behind the scalar queue
            xtc_all = {}
            for ki in range(FI_T):
                xtc = p.tile([128, NB], bf16, tag="xtc",
                             bufs=LR * FI_T, name=f"xtc0_{ki}")
                nc.sync.dma_start(xtc[:], xt[ki * 128:(ki + 1) * 128, 0:NB])
                xtc_all[(0, ki)] = xtc
            for ki in range(1, FI_T):
                nc.scalar.dma_start(w_c[:, ki * F:(ki + 1) * F],
                                    w[ki * 128:(ki + 1) * 128, :])
            # later ranks' X chunks also ride the sync queue: on the
            # scalar queue their DMA issues get serialized behind the H
            # evacuation copies, stalling the rank-2/3 H matmuls ~10us
            for rr in range(1, LR):
                for ki in range(FI_T):
                    xtc = p.tile([128, NB], bf16, tag="xtc",
                                 bufs=LR * FI_T, name=f"xtc{rr}_{ki}")
                    nc.sync.dma_start(
                        xtc[:],
                        xt[ki * 128:(ki + 1) * 128, rr * NB:(rr + 1) * NB])
                    xtc_all[(rr, ki)] = xtc

            cs = p.tile([128, KT], f32)    # per-core partial column sums
            ones = p.tile([128, NB], bf16)
            nc.vector.memset(ones[:], 1.0)
            ones1 = p.tile([1, 128], bf16)
            nc.vector.memset(ones1[:], 1.0)

            # single PSUM pool: 8 banks, all held by the main accumulators;
            # H compute reuses them as scratch (the first real matmul's
            # start=True clears each bank).
            pms = []
            for m in range(MT):
                pm = ps.tile([128, F], f32, tag=f"pm{m}", name=f"pm{m}",
                             bufs=1)
                pms.append(pm)

            # a few dep-free matmuls on `ones` ramp the PE p-state
            # (0.65->2.4GHz takes ~3us of continuous execution) while the
            # first X/W chunks are still in flight; banks 4..7 are reset
            # by the main loop's start=True matmuls.
            for j in range(4):
                nc.tensor.matmul(pms[4 + j][:], ones[:, 0:128],
                                 ones[:, 0:F], start=True, stop=True)

            # bias broadcast [128, F] built once via a K=1 matmul; H
            # evacuations add it on the DVE, replacing LR*8 per-tile
            # K=1 bias matmuls (~10us of PE).
            nc.tensor.matmul(pms[7][:], ones1[:], b_bf[:],
                             start=True, stop=True)
            b_bcast = p.tile([128, F], f32)
            nc.vector.tensor_copy(b_bcast[:], pms[7][:])

            # ---- A stream ----
            a_raws = []

            def emit_araw(i):
                kt_e = (pid * MT + i) % KT
                a_raw = p.tile([128, NB], bf16, tag="araw", bufs=RBUF,
                               name=f"araw{i}")
                nc.sync.dma_start(a_raw[:], at[bass.ds(kt_e * 128, 128), :])
                a_raws.append(a_raw)

            for i in range(RBUF):
                emit_araw(i)

            a_bins = []

            def emit_binarize(i):
                # one DVE op: a_bin = (a_raw != 0) * 1.0 (bf16, exact),
                # accum_out = free-dim sums = partial column sums of A.
                # cs is in ITERATION order (static AP keeps the dynamic-
                # offset register setup off the DVE hot path); the degree
                # combine below rotates per-rank slices to compensate.
                a_bin = p.tile([128, NB], bf16, tag="abin", bufs=ABUF,
                               name=f"abin{i}")
                nc.vector.scalar_tensor_tensor(
                    a_bin[:], a_raws[i][:], 0.0, ones[:],
                    mybir.AluOpType.not_equal, mybir.AluOpType.mult,
                    accum_out=cs[:, i:i + 1])
                a_bins.append(a_bin)

            # ---- H blocks for ranks pid..pid+LR-1 (bf16 matmuls) ----
            # bias lands via a K=1 matmul; evacuation is an ACT copy.
            hb_all = []
            for rr in range(LR):
                for nt in range(MT):
                    hp = pms[nt % 4]
                    for ki in range(FI_T):
                        nc.tensor.matmul(
                            hp[:],
                            xtc_all[(rr, ki)][:, nt * 128:(nt + 1) * 128],
                            w_c[:, ki * F:(ki + 1) * F],
                            start=(ki == 0), stop=(ki == FI_T - 1))
                    hb = p.tile([128, F], bf16, tag="hb", bufs=LR * MT,
                                name=f"hb{rr}_{nt}")
                    nc.vector.tensor_tensor(hb[:], hp[:], b_bcast[:],
                                            mybir.AluOpType.add)
                    if rr == 0:
                        nc.gpsimd.dma_start(
                            hg_in[nt * 128:(nt + 1) * 128, :], hb[:])
                    hb_all.append(hb)
                if rr == 0:
                    # issue the all-gather as soon as our own block is in
                    # hg_in; it runs while we compute H for the next ranks
                    # and chew through the local main-loop iterations.
                    nc.gpsimd.collective_compute(
                        "AllGather", mybir.AluOpType.bypass,
                        replica_groups=[list(range(N_CORES))],
                        ins=[hg_in.opt()], outs=[hg_out.opt()],
                    )
                # keep the DVE fed with early binarizes between H blocks
                for _ in range(PRE_BIN):
                    i = len(a_bins)
                    if i < KT:
                        emit_binarize(i)
                        if len(a_raws) < KT:
                            emit_araw(len(a_raws))

            # Main loop, rotated per core: iteration i handles physical tile
            # kt = (pid*8 + i) mod 64. The first LR*8 iterations use the
            # locally computed H blocks (no AllGather dependency); the rest
            # read the gathered hidden. PSUM accumulation is commutative.
            for i in range(KT):
                kt_e = (pid * MT + i) % KT
                if i >= len(a_bins):
                    emit_binarize(i)
                    if len(a_raws) < KT:
                        emit_araw(len(a_raws))
                if i < LR * MT:
                    rhs = hb_all[i][:]
                else:
                    # h_t DMAs live on the gpsimd queue WITH the
                    # all-gather: on the scalar queue they share a stream
                    # with the H evacuation copies, and the scheduler can
                    # order evacs behind them - a WAR convoy that blocks
                    # the first main matmul until the collective lands.
                    h_t = p.tile([128, F], bf16, tag="ht", bufs=HBUF,
                                 name=f"ht{i}")
                    nc.gpsimd.dma_start(h_t[:],
                                        hg_out[bass.ds(kt_e * 128, 128), :])
                    rhs = h_t[:]
                for m in range(MT):
                    nc.tensor.matmul(
                        pms[m][:],
                        a_bins[i][:, m * 128:(m + 1) * 128],
                        rhs,
                        start=(i == 0), stop=(i == KT - 1))

            # ---- phase 3: degree + normalize ----
            nc.sync.dma_start(dg_in[:], cs[:])
            nc.gpsimd.collective_compute(
                "AllGather", mybir.AluOpType.bypass,
                replica_groups=[list(range(N_CORES))],
                ins=[dg_in.opt()], outs=[dg_out.opt()],
            )
            # pull each rank's partial for OUR column block: rank r stores
            # kt=(r*8+i)%64 at iteration-column i, so our block (kt=pid*8+m)
            # sits at columns [((pid-r)%8)*8, +8) of rank r's slab
            deg = p.tile([128, MT], f32)
            prt0 = p.tile([128, MT], f32, tag="prt", bufs=4, name="prt0")
            nc.gpsimd.dma_start(prt0[:], dg_out[0:128, bass.ts(pid, MT)])
            nc.vector.tensor_copy(deg[:], prt0[:])
            for r in range(1, N_CORES):
                col = ((pid + (N_CORES - r)) % N_CORES) * MT
                prt = p.tile([128, MT], f32, tag="prt", bufs=4,
                             name=f"prt{r}")
                nc.gpsimd.dma_start(
                    prt[:],
                    dg_out[r * 128:(r + 1) * 128, bass.ds(col, MT)])
                nc.vector.tensor_tensor(deg[:], deg[:], prt[:],
                                        mybir.AluOpType.add)
            rdeg = p.tile([128, MT], f32)
            nc.vector.reciprocal(rdeg[:], deg[:])

            # normalize + store: each bank's out-DMA issues from the same
            # queue as its normalize op, avoiding a cross-queue semaphore
            # hop on the kernel tail.
            for m in range(MT):
                o_sb = p.tile([128, F], f32, tag="osb", bufs=4,
                              name=f"osb{m}")
                if m % 2 == 0:
                    nc.vector.tensor_scalar(o_sb[:], pms[m][:],
                                            rdeg[:, m:m + 1], None,
                                            mybir.AluOpType.mult)
                    nc.sync.dma_start(out[m * 128:(m + 1) * 128, :],
                                      o_sb[:])
                else:
                    nc.scalar.mul(o_sb[:], pms[m][:], rdeg[:, m:m + 1])
                    nc.scalar.dma_start(out[m * 128:(m + 1) * 128, :],
                                        o_sb[:])

    nc.compile()
    return nc


def _get_nc():
    if "nc" not in _cached:
        _cached["nc"] = _build()
    return _cached["nc"]


def kernel(input_features, adj, W, b):
    import ml_dtypes
    from concourse.bass_utils import run_bass_kernel_spmd

    bf16 = ml_dtypes.bfloat16
    x = np.asarray(input_features, dtype=np.float32)
    a = np.asarray(adj, dtype=np.float32)
    wm = np.ascontiguousarray(np.asarray(W, dtype=np.float32).astype(bf16))
    bv = np.ascontiguousarray(
        np.asarray(b, dtype=np.float32).astype(bf16).reshape(1, F))

    xts = [np.ascontiguousarray(x[k * NB:(k + 1) * NB, :].T.astype(bf16))
           for k in range(N_CORES)]

    nc = _get_nc()
    in_maps = []
    for k in range(N_CORES):
        blk = slice(k * NB, (k + 1) * NB)
        xt_cat = np.concatenate(
            [xts[(k + rr) % N_CORES] for rr in range(LR)], axis=1)
        in_maps.append({
            "at": np.ascontiguousarray(a[blk, :].T.astype(bf16)),
            "xt": np.ascontiguousarray(xt_cat),
            "w": wm,
            "bvec": bv,
        })
    res = run_bass_kernel_spmd(nc, in_maps, core_ids=list(range(N_CORES)))
    return np.concatenate([res.results[k]["out"] for k in range(N_CORES)],
                          axis=0)


# revision 45
# speedup vs baseline: 1.1924x; 1.1924x over previous
"""GCN layer kernel for Trainium2, 8-core SPMD.

Computes: out = (A @ (X @ W + b)) / colsum(A)[:, None],  A = (adj != 0)
with N=8192 nodes, F_in=F_out=512, across 8 NeuronCores.

Sharding: row-shard adjacency and node features (1024 rows per core),
replicate W/b. Degree (column sums of A) needs rows from every core: each
core computes partial column sums for free via the binarize op's
accumulate output, an all-gather shares them, and an on-device tree sum
+ reciprocal finishes the normalization.

All tensor inputs are shipped to HBM as bf16 (host-side dtype cast only:
the (adj != 0) predicate is exactly preserved by the cast since no
uniform[0,1) float rounds to 0 in bf16, and X/W/b are converted to bf16
on-device by the reference-equivalent compute path anyway). This halves
the dominant A-stream HBM traffic (32 -> 16 MB/core). Binarize + degree
+ projection + aggregation + normalization all still happen on device.

Measured-on-hw notes driving the structure:
- bf16 [128,512] matmuls: 454ns latency, ~265ns pipelined throughput
  (~1.95GHz effective). Main loop floor = 512 mm ~= 136us.
- The CC engine cannot enter a collective before ~20.5us and every
  collective pays a ~45us rendezvous after entry (NOT amortizable by a
  warm-up collective - that only queues ahead and delays the H gather).
  At ~160GB/s cc wire rate the H AllGather completes ~105-125us in.
- Each core computes H = X@W+b for itself and the next LR-1 ranks from
  cheap extra X inputs; the main loop visits contraction tiles in
  per-core rotated order kt=(pid*8+i)%64, so its first LR*8 iterations
  use local H blocks and cover the whole gather window; only the far
  tiles read gathered data.
- bias lands via a K=1 matmul per H tile; evacuation is an ACT copy.
  Keeping evacuations OFF the DVE matters: the static scheduler
  otherwise interleaves them with binarizes into a convoy that stalls
  the pipeline behind the collective.
- cs partials are stored in ITERATION order and bins/mms/araws keep a
  single monotonic order - reordering them (canonical columns,
  sub-major far order) makes the scheduler's coarsened semaphore waits
  couple PE matmuls to much-later binarizes, deadlocking the pipeline
  until the collective lands. The degree combine rotates per-rank
  slices to compensate.
"""
import numpy as np

N = 8192
F = 512
N_CORES = 8
NB = N // N_CORES          # 1024 rows per core
KT = N // 128              # 64 contraction tiles
MT = NB // 128             # 8 output row tiles per core
FI_T = F // 128            # 4 feat-in tiles
LR = 5                     # ranks whose H we compute locally
RBUF = 6                   # a_raw ring depth (DMA run-ahead)
ABUF = 30                  # a_bin ring depth (DVE run-ahead); deep so
                           # the binarizes (and the degree partials they
                           # accumulate) finish well before the PE does,
                           # keeping the degree AllGather off the tail
HBUF = 16                  # gathered-H tile ring depth
PRE_BIN = 3                # binarizes interleaved per H rank block

_cached = {}


def _build():
    import concourse.bacc as bacc
    import concourse.bass as bass
    import concourse.tile as tile
    from concourse import mybir

    f32 = mybir.dt.float32
    bf16 = mybir.dt.bfloat16

    nc = bacc.Bacc("TRN2", target_bir_lowering=False, debug=False,
                   num_devices=N_CORES)
    at = nc.dram_tensor("at", [N, NB], bf16, kind="ExternalInput").ap()
    xt = nc.dram_tensor("xt", [F, LR * NB], bf16, kind="ExternalInput").ap()
    w = nc.dram_tensor("w", [F, F], bf16, kind="ExternalInput").ap()
    bvec = nc.dram_tensor("bvec", [1, F], bf16, kind="ExternalInput").ap()
    out = nc.dram_tensor("out", [NB, F], f32, kind="ExternalOutput").ap()

    pid = nc.partition_id()

    with tile.TileContext(nc) as tc:
        with tc.tile_pool(name="dram", bufs=1, space="DRAM") as dram, \
             tc.tile_pool(name="p", bufs=1) as p, \
             tc.tile_pool(name="ps", bufs=1, space="PSUM") as ps:
            hg_in = dram.tile([NB, F], bf16)
            hg_out = dram.tile([N, F], bf16, addr_space="Shared")
            dg_in = dram.tile([128, KT], f32)
            dg_out = dram.tile([128 * N_CORES, KT], f32, addr_space="Shared")

            # ---- critical-path DMAs first: b, W chunk 0, rank-0 X chunks
            b_bf = p.tile([1, F], bf16)
            nc.scalar.dma_start(b_bf[:], bvec)
            w_c = p.tile([128, FI_T * F], bf16)
            nc.scalar.dma_start(w_c[:, 0:F], w[0:128, :])
            # rank-0 X chunks ride the (otherwise idle) sync queue so the
            # first H matmul is not serialized behind the scalar queue
            xtc_all = {}
            for ki in range(FI_T):
                xtc = p.tile([128, NB], bf16, tag="xtc",
                             bufs=LR * FI_T, name=f"xtc0_{ki}")
                nc.sync.dma_start(xtc[:], xt[ki * 128:(ki + 1) * 128, 0:NB])
                xtc_all[(0, ki)] = xtc
            for ki in range(1, FI_T):
                nc.scalar.dma_start(w_c[:, ki * F:(ki + 1) * F],
                                    w[ki * 128:(ki + 1) * 128, :])
            # later ranks' X chunks also ride the sync queue: on the
            # scalar queue their DMA issues get serialized behind the H
            # evacuation copies, stalling the rank-2/3 H matmuls ~10us
            for rr in range(1, LR):
                for ki in range(FI_T):
                    xtc = p.tile([128, NB], bf16, tag="xtc",
                                 bufs=LR * FI_T, name=f"xtc{rr}_{ki}")
                    nc.sync.dma_start(
                        xtc[:],
                        xt[ki * 128:(ki + 1) * 128, rr * NB:(rr + 1) * NB])
                    xtc_all[(rr, ki)] = xtc

            cs = p.tile([128, KT], f32)    # per-core partial column sums
            ones = p.tile([128, NB], bf16)
            nc.vector.memset(ones[:], 1.0)
            ones1 = p.tile([1, 128], bf16)
            nc.vector.memset(ones1[:], 1.0)

            # single PSUM pool: 8 banks, all held by the main accumulators;
            # H compute reuses them as scratch (the first real matmul's
            # start=True clears each bank).
            pms = []
            for m in range(MT):
                pm = ps.tile([128, F], f32, tag=f"pm{m}", name=f"pm{m}",
                             bufs=1)
                pms.append(pm)

            # a few dep-free matmuls on `ones` ramp the PE p-state
            # (0.65->2.4GHz takes ~3us of continuous execution) while the
            # first X/W chunks are still in flight; banks 4..7 are reset
            # by the main loop's start=True matmuls.
            for j in range(4):
                nc.tensor.matmul(pms[4 + j][:], ones[:, 0:128],
                                 ones[:, 0:F], start=True, stop=True)

            # ---- A stream ----
            a_raws = []

            def emit_araw(i):
                kt_e = (pid * MT + i) % KT
                a_raw = p.tile([128, NB], bf16, tag="araw", bufs=RBUF,
                               name=f"araw{i}")
                nc.sync.dma_start(a_raw[:], at[bass.ds(kt_e * 128, 128), :])
                a_raws.append(a_raw)

            for i in range(RBUF):
                emit_araw(i)

            a_bins = []

            def emit_binarize(i):
                # one DVE op: a_bin = (a_raw != 0) * 1.0 (bf16, exact),
                # accum_out = free-dim sums = partial column sums of A.
                # cs is in ITERATION order (static AP keeps the dynamic-
                # offset register setup off the DVE hot path); the degree
                # combine below rotates per-rank slices to compensate.
                a_bin = p.tile([128, NB], bf16, tag="abin", bufs=ABUF,
                               name=f"abin{i}")
                nc.vector.scalar_tensor_tensor(
                    a_bin[:], a_raws[i][:], 0.0, ones[:],
                    mybir.AluOpType.not_equal, mybir.AluOpType.mult,
                    accum_out=cs[:, i:i + 1])
                a_bins.append(a_bin)

            # ---- H blocks for ranks pid..pid+LR-1 (bf16 matmuls) ----
            # bias lands via a K=1 matmul; evacuation is an ACT copy.
            hb_all = []
            for rr in range(LR):
                for nt in range(MT):
                    hp = pms[nt % 4]
                    for ki in range(FI_T):
                        nc.tensor.matmul(
                            hp[:],
                            xtc_all[(rr, ki)][:, nt * 128:(nt + 1) * 128],
                            w_c[:, ki * F:(ki + 1) * F],
                            start=(ki == 0), stop=False)
                    nc.tensor.matmul(hp[:], ones1[:], b_bf[:],
                                     start=False, stop=True)
                    hb = p.tile([128, F], bf16, tag="hb", bufs=LR * MT,
                                name=f"hb{rr}_{nt}")
                    nc.scalar.copy(hb[:], hp[:])
                    if rr == 0:
                        nc.gpsimd.dma_start(
                            hg_in[nt * 128:(nt + 1) * 128, :], hb[:])
                    hb_all.append(hb)
                if rr == 0:
                    # issue the all-gather as soon as our own block is in
                    # hg_in; it runs while we compute H for the next ranks
                    # and chew through the local main-loop iterations.
                    nc.gpsimd.collective_compute(
                        "AllGather", mybir.AluOpType.bypass,
                        replica_groups=[list(range(N_CORES))],
                        ins=[hg_in.opt()], outs=[hg_out.opt()],
                    )
                # keep the DVE fed with early binarizes between H blocks
                for _ in range(PRE_BIN):
                    i = len(a_bins)
                    if i < KT:
                        emit_binarize(i)
                        if len(a_raws) < KT:
                            emit_araw(len(a_raws))

            # Main loop, rotated per core: iteration i handles physical tile
            # kt = (pid*8 + i) mod 64. The first LR*8 iterations use the
            # locally computed H blocks (no AllGather dependency); the rest
            # read the gathered hidden. PSUM accumulation is commutative.
            for i in range(KT):
                kt_e = (pid * MT + i) % KT
                if i >= len(a_bins):
                    emit_binarize(i)
                    if len(a_raws) < KT:
                        emit_araw(len(a_raws))
                if i < LR * MT:
                    rhs = hb_all[i][:]
                else:
                    # h_t DMAs live on the gpsimd queue WITH the
                    # all-gather: on the scalar queue they share a stream
                    # with the H evacuation copies, and the scheduler can
                    # order evacs behind them - a WAR convoy that blocks
                    # the first main matmul until the collective lands.
                    h_t = p.tile([128, F], bf16, tag="ht", bufs=HBUF,
                                 name=f"ht{i}")
                    nc.gpsimd.dma_start(h_t[:],
                                        hg_out[bass.ds(kt_e * 128, 128), :])
                    rhs = h_t[:]
                for m in range(MT):
                    nc.tensor.matmul(
                        pms[m][:],
                        a_bins[i][:, m * 128:(m + 1) * 128],
                        rhs,
                        start=(i == 0), stop=(i == KT - 1))

            # ---- phase 3: degree + normalize ----
            nc.sync.dma_start(dg_in[:], cs[:])
            nc.gpsimd.collective_compute(
                "AllGather", mybir.AluOpType.bypass,
                replica_groups=[list(range(N_CORES))],
                ins=[dg_in.opt()], outs=[dg_out.opt()],
            )
            # pull each rank's partial for OUR column block: rank r stores
            # kt=(r*8+i)%64 at iteration-column i, so our block (kt=pid*8+m)
            # sits at columns [((pid-r)%8)*8, +8) of rank r's slab
            deg = p.tile([128, MT], f32)
            prt0 = p.tile([128, MT], f32, tag="prt", bufs=4, name="prt0")
            nc.gpsimd.dma_start(prt0[:], dg_out[0:128, bass.ts(pid, MT)])
            nc.vector.tensor_copy(deg[:], prt0[:])
            for r in range(1, N_CORES):
                col = ((pid + (N_CORES - r)) % N_CORES) * MT
                prt = p.tile([128, MT], f32, tag="prt", bufs=4,
                             name=f"prt{r}")
                nc.gpsimd.dma_start(
                    prt[:],
                    dg_out[r * 128:(r + 1) * 128, bass.ds(col, MT)])
                nc.vector.tensor_tensor(deg[:], deg[:], prt[:],
                                        mybir.AluOpType.add)
            rdeg = p.tile([128, MT], f32)
            nc.vector.reciprocal(rdeg[:], deg[:])

            # normalize + store: each bank's out-DMA issues from the same
            # queue as its normalize op, avoiding a cross-queue semaphore
            # hop on the kernel tail.
            for m in range(MT):
                o_sb = p.tile([128, F], f32, tag="osb", bufs=4,
                              name=f"osb{m}")
                if m % 2 == 0:
                    nc.vector.tensor_scalar(o_sb[:], pms[m][:],
                                            rdeg[:, m:m + 1], None,
                                            mybir.AluOpType.mult)
                    nc.sync.dma_start(out[m * 128:(m + 1) * 128, :],
                                      o_sb[:])
                else:
                    nc.scalar.mul(o_sb[:], pms[m][:], rdeg[:, m:m + 1])
                    nc.scalar.dma_start(out[m * 128:(m + 1) * 128, :],
                                        o_sb[:])

    nc.compile()
    return nc


def _get_nc():
    if "nc" not in _cached:
        _cached["nc"] = _build()
    return _cached["nc"]


def kernel(input_features, adj, W, b):
    import ml_dtypes
    from concourse.bass_utils import run_bass_kernel_spmd

    bf16 = ml_dtypes.bfloat16
    x = np.asarray(input_features, dtype=np.float32)
    a = np.asarray(adj, dtype=np.float32)
    wm = np.ascontiguousarray(np.asarray(W, dtype=np.float32).astype(bf16))
    bv = np.ascontiguousarray(
        np.asarray(b, dtype=np.float32).astype(bf16).reshape(1, F))

    xts = [np.ascontiguousarray(x[k * NB:(k + 1) * NB, :].T.astype(bf16))
           for k in range(N_CORES)]

    nc = _get_nc()
    in_maps = []
    for k in range(N_CORES):
        blk = slice(k * NB, (k + 1) * NB)
        xt_cat = np.concatenate(
            [xts[(k + rr) % N_CORES] for rr in range(LR)], axis=1)
        in_maps.append({
            "at": np.ascontiguousarray(a[blk, :].T.astype(bf16)),
            "xt": np.ascontiguousarray(xt_cat),
            "w": wm,
            "bvec": bv,
        })
    res = run_bass_kernel_spmd(nc, in_maps, core_ids=list(range(N_CORES)))
    return np.concatenate([res.results[k]["out"] for k in range(N_CORES)],
                          axis=0)
